# revision 14
# baseline (speedup 1.0000x reference)
"""AKT (knowledge-tracing transformer) forward pass on 8 TRN2 NeuronCores.

Sharding: pure data-parallel over batch (bs=8 -> 1 row/core). All params
replicated. No collectives; the final masked BCE sum is reduced on host
from per-core partial results.

Per-core kernel layout choice: activations live TRANSPOSED in SBUF
(d_model on partitions, seqlen on free dim), so projections/FFN are
direct TensorEngine matmuls (lhsT = weight as stored). LayerNorm stats
(partition-axis sums) are computed with ones-vector matmuls, and
broadcast back over partitions with K=1 matmuls.
"""

import math
import os
import sys

import numpy as np

for _p in ("/opt/trn_rl_repo",):
    if _p not in sys.path:
        sys.path.insert(0, _p)

from contextlib import ExitStack

import concourse.bass as bass
import concourse.mybir as mybir
import concourse.tile as tile
from concourse import bacc
from concourse.bass_utils import run_bass_kernel_spmd
from concourse.masks import make_identity

FP = mybir.dt.float32
I32 = mybir.dt.int32
AF = mybir.ActivationFunctionType
ALU = mybir.AluOpType
AX = mybir.AxisListType

S = 512          # seqlen
D = 256          # d_model
H = 8            # heads
DK = 32          # head dim
DFF = 2048
NQ = 10000
FC1N = 512
FC2N = 256
P = 128
NEG = -1.0e32
NKC = D // P     # 2 chunks of d_model on partitions
NST = S // P     # 4 seq tiles
NCORES = 8

LAST_RESULT = None   # BassKernelResults of the most recent run (for profiling)
_NC_CACHE = {}


def _build():
    nc = bacc.Bacc(None, target_bir_lowering=False)
    h = {}

    def din(name, shape, dtype=FP):
        t = nc.dram_tensor(name, list(shape), dtype, kind="ExternalInput")
        h[name] = t
        return t

    din("q_idx", (S, 1), I32)
    din("r_idx", (S, 1), I32)
    din("g_idx", (S, 1), I32)
    din("tgt", (1, S))
    din("q_embed", (NQ + 1, D))
    din("qa_embed", (2, D))
    din("graph_embed", (100 * S + 1, D))
    din("posT", (D, S))
    din("mask1", (S, S))
    din("mask0", (S, S))
    din("posabs", (S, S))
    for si in range(3):
        for w in ("wq", "wk", "wv", "wo"):
            din(f"{w}{si}", (D, D))
    for si in (0, 2):
        din(f"w1{si}", (D, DFF))
        din(f"w2{si}", (DFF, D))
    din("fc1", (2 * D, FC1N))
    din("fc2", (FC1N, FC2N))
    din("fc3", (FC2N, 1))
    din("gneg", (P, 24))
    out_h = nc.dram_tensor("out", [2, S], FP, kind="ExternalOutput")

    with tile.TileContext(nc) as tc, ExitStack() as ctx:
        pc = ctx.enter_context(tc.tile_pool(name="consts", bufs=1))
        pw = ctx.enter_context(tc.tile_pool(name="wts", bufs=1))
        pa = ctx.enter_context(tc.tile_pool(name="acts", bufs=1))
        pt = ctx.enter_context(tc.tile_pool(name="tmp", bufs=1))
        pp = ctx.enter_context(tc.tile_pool(name="ps", bufs=1, space="PSUM"))

        def ptile(nm, tag, bufs, shape=(P, S), dt=FP, pool=None):
            pool = pool or pt
            return pool.tile(list(shape), dt, name=nm, tag=tag, bufs=bufs)

        def pstile(nm, tag, bufs, shape=(P, S)):
            return pp.tile(list(shape), FP, name=nm, tag=tag, bufs=bufs,
                           space="PSUM")

        # ---- constants ----
        ident = pc.tile([P, P], FP, name="ident", tag="ident", bufs=1)
        make_identity(nc, ident[:])
        ones = pc.tile([P, P], FP, name="onesc", tag="onesc", bufs=1)
        nc.gpsimd.memset(ones[:], 1.0)
        eps_sb = pc.tile([1, 1], FP, name="epssb", tag="epssb", bufs=1)
        nc.gpsimd.memset(eps_sb[:], 1e-5)
        gneg_sb = pc.tile([P, 24], FP, name="gnegsb", tag="gnegsb", bufs=1)
        nc.sync.dma_start(out=gneg_sb[:], in_=h["gneg"][:, :])
        tgt_sb = pc.tile([1, S], FP, name="tgtsb", tag="tgtsb", bufs=1)
        nc.sync.dma_start(out=tgt_sb[:], in_=h["tgt"][:, :])

        mask_sb = {}
        for flag, nmm in ((1, "mask1"), (0, "mask0")):
            tl = []
            for i in range(NST):
                t = pc.tile([P, S], FP, name=f"{nmm}_{i}", tag=f"{nmm}_{i}",
                            bufs=1)
                nc.sync.dma_start(out=t[:], in_=h[nmm][i * P:(i + 1) * P, :])
                tl.append(t)
            mask_sb[flag] = tl
        pos_sb = []
        for i in range(NST):
            t = pc.tile([P, S], FP, name=f"pos_{i}", tag=f"pos_{i}", bufs=1)
            nc.sync.dma_start(out=t[:], in_=h["posabs"][i * P:(i + 1) * P, :])
            pos_sb.append(t)
        posT_sb = []
        for k in range(NKC):
            t = pc.tile([P, S], FP, name=f"posT_{k}", tag=f"posT_{k}", bufs=1)
            nc.sync.dma_start(out=t[:], in_=h["posT"][k * P:(k + 1) * P, :])
            posT_sb.append(t)

        # ---- embedding gathers (indirect DMA) + transpose to T layout ----
        def gather_T(nm, table_h, idx_h, tag, bufs, evac_adds=None):
            nat = []
            for i in range(NST):
                ix = ptile(f"{nm}ix{i}", "idx", 4, (P, 1), I32)
                nc.sync.dma_start(out=ix[:], in_=idx_h[i * P:(i + 1) * P, :])
                en = ptile(f"{nm}nat{i}", "enat", 4, (P, D))
                nc.gpsimd.indirect_dma_start(
                    out=en[:], out_offset=None, in_=table_h[:, :],
                    in_offset=bass.IndirectOffsetOnAxis(ap=ix[:, :1], axis=0))
                nat.append(en)
            eT = []
            for k in range(NKC):
                ps = pstile(f"{nm}psT{k}", "ps_t", 2)
                for i in range(NST):
                    nc.tensor.transpose(out=ps[:, i * P:(i + 1) * P],
                                        in_=nat[i][:, k * P:(k + 1) * P],
                                        identity=ident[:])
                o = ptile(f"{nm}T{k}", tag, bufs, pool=pa)
                if evac_adds is None:
                    nc.scalar.copy(out=o[:], in_=ps[:])
                else:
                    nc.vector.tensor_add(out=o[:], in0=ps[:],
                                         in1=evac_adds[0][k][:])
                    for extra in evac_adds[1:]:
                        nc.vector.tensor_add(out=o[:], in0=o[:],
                                             in1=extra[k][:])
                eT.append(o)
            return eT

        qeT = gather_T("qe", h["q_embed"], h["q_idx"], "qeT", 2)
        gT = gather_T("gg", h["graph_embed"], h["g_idx"], "gT", 2)
        xT0 = []
        for k in range(NKC):
            o = ptile(f"x0_{k}", "xio", 8, pool=pa)
            nc.vector.tensor_add(out=o[:], in0=qeT[k][:], in1=posT_sb[k][:])
            xT0.append(o)
        yT0 = gather_T("qa", h["qa_embed"], h["r_idx"], "xio", 8,
                       evac_adds=[qeT, posT_sb])

        # ---- building blocks ----
        def load_w(name_dram, tag, ncols):
            w_sb = []
            for k in range(NKC):
                t = ptile(f"{tag}_{k}_{nc.next_id()}", tag, 2, (P, ncols),
                          pool=pw)
                nc.sync.dma_start(out=t[:],
                                  in_=h[name_dram][k * P:(k + 1) * P, :])
                w_sb.append(t)
            return w_sb

        def proj_T(xTs, w_sb, out_tag, out_bufs=2, evac="copy", res_in=None):
            outs = []
            for m in range(NKC):
                ps = pstile(f"pj{out_tag}{m}_{nc.next_id()}", "ps_mm", 3)
                for k in range(NKC):
                    nc.tensor.matmul(out=ps[:],
                                     lhsT=w_sb[k][:, m * P:(m + 1) * P],
                                     rhs=xTs[k][:],
                                     start=(k == 0), stop=(k == NKC - 1))
                o = ptile(f"{out_tag}o{m}_{nc.next_id()}", out_tag, out_bufs,
                          pool=pa)
                if evac == "copy":
                    nc.scalar.copy(out=o[:], in_=ps[:])
                else:  # residual add
                    nc.vector.tensor_add(out=o[:], in0=ps[:],
                                         in1=res_in[m][:])
                outs.append(o)
            return outs

        def layernorm_T(xs, out_tag, out_bufs):
            sq = []
            for k in range(NKC):
                sqk = ptile(f"sq{k}_{nc.next_id()}", "sq", 2)
                nc.scalar.activation(out=sqk[:], in_=xs[k][:], func=AF.Square)
                sq.append(sqk)
            st = pstile(f"lnst_{nc.next_id()}", "ps_st", 1, (64, S))
            for k in range(NKC):
                nc.tensor.matmul(out=st[0:1, :], lhsT=ones[:, 0:1],
                                 rhs=xs[k][:], start=(k == 0),
                                 stop=(k == NKC - 1), tile_position=(0, 0))
            for k in range(NKC):
                nc.tensor.matmul(out=st[32:33, :], lhsT=ones[:, 0:1],
                                 rhs=sq[k][:], start=(k == 0),
                                 stop=(k == NKC - 1), tile_position=(0, 32))
            mu = ptile(f"mu_{nc.next_id()}", "lns", 2, (1, S))
            nc.vector.tensor_scalar_mul(out=mu[:], in0=st[0:1, :],
                                        scalar1=1.0 / D)
            ex2 = ptile(f"ex2_{nc.next_id()}", "lns", 2, (1, S))
            nc.vector.tensor_scalar_mul(out=ex2[:], in0=st[32:33, :],
                                        scalar1=1.0 / D)
            msq = ptile(f"msq_{nc.next_id()}", "lns1", 4, (1, S))
            nc.scalar.activation(out=msq[:], in_=mu[:], func=AF.Square)
            var = ptile(f"var_{nc.next_id()}", "lns1", 4, (1, S))
            nc.vector.tensor_tensor(out=var[:], in0=ex2[:], in1=msq[:],
                                    op=ALU.subtract)
            # rstd = exp(-0.5*ln(var+eps)) — avoids Sqrt's ACT table set
            lv = ptile(f"lv_{nc.next_id()}", "lns1", 4, (1, S))
            nc.scalar.activation(out=lv[:], in_=var[:], func=AF.Ln,
                                 bias=eps_sb[0:1, 0:1])
            rstd = ptile(f"rstd_{nc.next_id()}", "lns1", 4, (1, S))
            nc.scalar.activation(out=rstd[:], in_=lv[:], func=AF.Exp,
                                 scale=-0.5)
            mrs = ptile(f"mrs_{nc.next_id()}", "lns1", 4, (1, S))
            nc.vector.tensor_tensor(out=mrs[:], in0=mu[:], in1=rstd[:],
                                    op=ALU.mult)
            A = pstile(f"lnA_{nc.next_id()}", "ps_t", 2)
            nc.tensor.matmul(out=A[:], lhsT=ones[0:1, :], rhs=rstd[:],
                             start=True, stop=True)
            B = pstile(f"lnB_{nc.next_id()}", "ps_av", 2)
            nc.tensor.matmul(out=B[:], lhsT=ones[0:1, :], rhs=mrs[:],
                             start=True, stop=True)
            outs = []
            for k in range(NKC):
                o = ptile(f"{out_tag}ln{k}_{nc.next_id()}", out_tag, out_bufs,
                          pool=pa)
                nc.vector.tensor_tensor(out=o[:], in0=xs[k][:], in1=A[:],
                                        op=ALU.mult)
                nc.vector.tensor_tensor(out=o[:], in0=o[:], in1=B[:],
                                        op=ALU.subtract)
                outs.append(o)
            return outs

        def attention(QT, KT, VT, flag, set_idx):
            # V back to natural layout (seq on partitions) for attn@V
            vnat = []
            for j in range(NST):
                ps = pstile(f"vn{j}_{nc.next_id()}", "ps_t", 2, (P, D))
                for k in range(NKC):
                    nc.tensor.transpose(out=ps[:, k * P:(k + 1) * P],
                                        in_=VT[k][:, j * P:(j + 1) * P],
                                        identity=ident[:])
                o = ptile(f"vnat{j}_{nc.next_id()}", "vnat", 4, (P, D))
                nc.scalar.copy(out=o[:], in_=ps[:])
                vnat.append(o)
            msk = mask_sb[flag]
            aoT = []
            for hg in range(2):
                av = pstile(f"av{hg}_{nc.next_id()}", "ps_av", 2)
                for hh in range(4):
                    hd = hg * 4 + hh
                    gcol = set_idx * 8 + hd
                    attn_tiles = []
                    for i in range(NST):
                        uid = nc.next_id()
                        sc_ps = pstile(f"sc{uid}", "ps_mm", 3)
                        nc.tensor.matmul(
                            out=sc_ps[:],
                            lhsT=QT[hg][32 * hh:32 * hh + 32, i * P:(i + 1) * P],
                            rhs=KT[hg][32 * hh:32 * hh + 32, :],
                            start=True, stop=True,
                            tile_position=(32 * hh, 0))
                        masked = ptile(f"mk{uid}", "mu", 2)
                        nc.vector.tensor_add(out=masked[:], in0=sc_ps[:],
                                             in1=msk[i][:])
                        nm1 = ptile(f"nm1{uid}", "nm", 3, (P, 1))
                        nc.vector.tensor_reduce(out=nm1[:], in_=masked[:],
                                                axis=AX.X, op=ALU.max,
                                                negate=True)
                        e1 = ptile(f"e1{uid}", "e", 2)
                        z1 = ptile(f"z1{uid}", "z", 3, (P, 1))
                        nc.scalar.activation(out=e1[:], in_=masked[:],
                                             func=AF.Exp, bias=nm1[:, 0:1],
                                             accum_out=z1[:, 0:1])
                        dc = ptile(f"dc{uid}", "dc", 2)
                        nc.vector.tensor_tensor_scan(
                            out=dc[:], data0=e1[:], data1=e1[:], initial=0.0,
                            op0=ALU.add, op1=ALU.bypass)
                        # -(max(z1-cumsum,0)) = min(dc,z1)-z1
                        an = ptile(f"an{uid}", "an", 3)
                        nc.vector.tensor_scalar(out=an[:], in0=dc[:],
                                                scalar1=z1[:, 0:1],
                                                scalar2=z1[:, 0:1],
                                                op0=ALU.min, op1=ALU.subtract)
                        un = ptile(f"un{uid}", "an", 3)
                        nc.vector.tensor_tensor(out=un[:], in0=an[:],
                                                in1=pos_sb[i][:], op=ALU.mult)
                        # sqrt via exp/ln to stay in one ACT table set:
                        # dist*gamma/sqrt(z1) needs sqrt(un) * g * rsqrt(z1)
                        lu = ptile(f"lu{uid}", "an", 3)
                        nc.scalar.activation(out=lu[:], in_=un[:],
                                             func=AF.Ln, scale=-1.0)
                        sq = ptile(f"sq{uid}", "an", 3)
                        nc.scalar.activation(out=sq[:], in_=lu[:],
                                             func=AF.Exp, scale=0.5)
                        lz = ptile(f"lz{uid}", "rz", 3, (P, 1))
                        nc.scalar.activation(out=lz[:], in_=z1[:, 0:1],
                                             func=AF.Ln)
                        srz = ptile(f"srz{uid}", "srz", 3, (P, 1))
                        nc.scalar.activation(out=srz[:], in_=lz[:, 0:1],
                                             func=AF.Exp, scale=-0.5)
                        gr = ptile(f"gr{uid}", "gr", 3, (P, 1))
                        nc.vector.tensor_scalar_mul(
                            out=gr[:], in0=srz[:, 0:1],
                            scalar1=gneg_sb[:, gcol:gcol + 1])
                        te = ptile(f"te{uid}", "an", 3)
                        nc.scalar.activation(out=te[:], in_=sq[:],
                                             func=AF.Exp, scale=gr[:, 0:1])
                        u2 = ptile(f"u2{uid}", "mu", 2)
                        nc.vector.tensor_tensor(out=u2[:], in0=sc_ps[:],
                                                in1=te[:], op=ALU.mult)
                        nm2 = ptile(f"nm2{uid}", "nm", 3, (P, 1))
                        nc.vector.tensor_reduce(out=nm2[:], in_=u2[:],
                                                axis=AX.X, op=ALU.max,
                                                negate=True)
                        e2 = ptile(f"e2{uid}", "e", 2)
                        z2 = ptile(f"z2{uid}", "z", 3, (P, 1))
                        nc.scalar.activation(out=e2[:], in_=u2[:],
                                             func=AF.Exp, bias=nm2[:, 0:1],
                                             accum_out=z2[:, 0:1])
                        r2 = ptile(f"r2{uid}", "rz", 3, (P, 1))
                        nc.vector.reciprocal(out=r2[:], in_=z2[:, 0:1])
                        at = ptile(f"at{uid}", "attn", 5)
                        nc.vector.tensor_scalar_mul(out=at[:], in0=e2[:],
                                                    scalar1=r2[:, 0:1])
                        if flag == 0 and i == 0:
                            nc.gpsimd.memset(at[0:1, :], 0.0)
                        attn_tiles.append(at)
                    for j in range(NST):
                        uid = nc.next_id()
                        tp = pstile(f"tp{uid}", "ps_t", 2)
                        for i in range(NST):
                            nc.tensor.transpose(
                                out=tp[:, i * P:(i + 1) * P],
                                in_=attn_tiles[i][:, j * P:(j + 1) * P],
                                identity=ident[:])
                        atT = ptile(f"atT{uid}", "attnT", 2)
                        nc.scalar.copy(out=atT[:], in_=tp[:])
                        nc.tensor.matmul(
                            out=av[32 * hh:32 * hh + 32, :],
                            lhsT=vnat[j][:, hd * DK:(hd + 1) * DK],
                            rhs=atT[:],
                            start=(j == 0), stop=(j == NST - 1),
                            tile_position=(0, 32 * hh))
                o = ptile(f"aoT{hg}_{nc.next_id()}", "aoT", 2, pool=pa)
                nc.scalar.copy(out=o[:], in_=av[:])
                aoT.append(o)
            return aoT

        def ffn(xs, set_idx):
            # w1 streamed as (P, 512) column groups, w2 as (P, D) row chunks
            w1_sb = {}
            for g in range(4):
                for k in range(NKC):
                    t = ptile(f"w1_{k}_{g}_{nc.next_id()}", "w1", 4, (P, 512),
                              pool=pw)
                    nc.sync.dma_start(
                        out=t[:],
                        in_=h[f"w1{set_idx}"][k * P:(k + 1) * P,
                                              g * 512:(g + 1) * 512])
                    w1_sb[(k, g)] = t
            w2_sb = []
            for k in range(DFF // P):
                t = ptile(f"w2_{k}_{nc.next_id()}", "w2", 6, (P, D), pool=pw)
                nc.sync.dma_start(
                    out=t[:], in_=h[f"w2{set_idx}"][k * P:(k + 1) * P, :])
                w2_sb.append(t)
            h2_ps = [pstile(f"h2ps{m}_{nc.next_id()}", "ps_av", 2)
                     for m in range(NKC)]
            for kk in range(DFF // P):
                g, loc = kk // 4, kk % 4
                h1_ps = pstile(f"h1ps{kk}_{nc.next_id()}", "ps_mm", 3)
                for k in range(NKC):
                    nc.tensor.matmul(out=h1_ps[:],
                                     lhsT=w1_sb[(k, g)][:, loc * P:(loc + 1) * P],
                                     rhs=xs[k][:], start=(k == 0),
                                     stop=(k == NKC - 1))
                h1 = ptile(f"h1_{kk}_{nc.next_id()}", "h1", 3)
                nc.scalar.activation(out=h1[:], in_=h1_ps[:], func=AF.Relu)
                for m in range(NKC):
                    nc.tensor.matmul(out=h2_ps[m][:],
                                     lhsT=w2_sb[kk][:, m * P:(m + 1) * P],
                                     rhs=h1[:], start=(kk == 0),
                                     stop=(kk == DFF // P - 1))
            res2 = []
            for m in range(NKC):
                r = ptile(f"res2_{m}_{nc.next_id()}", "res", 3)
                nc.vector.tensor_add(out=r[:], in0=h2_ps[m][:], in1=xs[m][:])
                res2.append(r)
            return res2

        def tlayer(set_idx, flag, qT, kT, vT, apply_ffn):
            wq = load_w(f"wq{set_idx}", "wq", D)
            wk = load_w(f"wk{set_idx}", "wk", D)
            wv = load_w(f"wv{set_idx}", "wv", D)
            QT = proj_T(qT, wq, "qt")
            KT = proj_T(kT, wk, "kt")
            VT = proj_T(vT, wv, "vt")
            aoT = attention(QT, KT, VT, flag, set_idx)
            wo = load_w(f"wo{set_idx}", "wo", D)
            res = proj_T(aoT, wo, "res", out_bufs=3, evac="res", res_in=qT)
            x1 = layernorm_T(res, "xio", 8)
            if not apply_ffn:
                return x1
            res2 = ffn(x1, set_idx)
            return layernorm_T(res2, "xio", 8)

        def addg(xs, nm):
            outs = []
            for k in range(NKC):
                o = ptile(f"{nm}{k}_{nc.next_id()}", "xio", 8, pool=pa)
                nc.vector.tensor_add(out=o[:], in0=xs[k][:], in1=gT[k][:])
                outs.append(o)
            return outs

        y1 = tlayer(0, 1, yT0, yT0, yT0, True)
        x1 = tlayer(1, 1, xT0, xT0, xT0, False)
        x2 = tlayer(2, 0, x1, x1, y1, True)
        xg = addg(x2, "xg")
        x3 = tlayer(1, 1, xg, xg, xg, False)
        xg2 = addg(x3, "xh")
        x4 = tlayer(2, 0, xg2, xg2, x3, True)

        # ---- output head: c = [x4, q_e]; 3 FCs; sigmoid + bce ----
        cT = [x4[0], x4[1], qeT[0], qeT[1]]
        fc1_sb = []
        for k in range(4):
            t = ptile(f"fc1_{k}", "fc1", 4, (P, FC1N), pool=pw)
            nc.sync.dma_start(out=t[:], in_=h["fc1"][k * P:(k + 1) * P, :])
            fc1_sb.append(t)
        f1 = []
        for m in range(4):
            ps = pstile(f"f1ps{m}", "ps_mm", 3)
            for k in range(4):
                nc.tensor.matmul(out=ps[:],
                                 lhsT=fc1_sb[k][:, m * P:(m + 1) * P],
                                 rhs=cT[k][:], start=(k == 0), stop=(k == 3))
            t = ptile(f"f1_{m}", "f1", 4)
            nc.scalar.activation(out=t[:], in_=ps[:], func=AF.Relu)
            f1.append(t)
        fc2_sb = []
        for k in range(4):
            t = ptile(f"fc2_{k}", "fc2", 4, (P, FC2N), pool=pw)
            nc.sync.dma_start(out=t[:], in_=h["fc2"][k * P:(k + 1) * P, :])
            fc2_sb.append(t)
        f2 = []
        for m in range(NKC):
            ps = pstile(f"f2ps{m}", "ps_av", 2)
            for k in range(4):
                nc.tensor.matmul(out=ps[:],
                                 lhsT=fc2_sb[k][:, m * P:(m + 1) * P],
                                 rhs=f1[k][:], start=(k == 0), stop=(k == 3))
            t = ptile(f"f2_{m}", "h1", 3)
            nc.scalar.activation(out=t[:], in_=ps[:], func=AF.Relu)
            f2.append(t)
        fc3_sb = []
        for k in range(NKC):
            t = ptile(f"fc3_{k}", "fc3", 2, (P, 1), pool=pw)
            nc.sync.dma_start(out=t[:], in_=h["fc3"][k * P:(k + 1) * P, :])
            fc3_sb.append(t)
        pps = pstile("predps", "ps_mm", 3, (1, S))
        for k in range(NKC):
            nc.tensor.matmul(out=pps[:], lhsT=fc3_sb[k][:, 0:1],
                             rhs=f2[k][:], start=(k == 0), stop=(k == 1))
        out_sb = ptile("outsb", "outsb", 1, (1, 2 * S))
        # sigmoid(p) = 1/(1+exp(-p)) — keeps the single exp/ln ACT table
        en = ptile("ben", "lns1", 4, (1, S))
        nc.scalar.activation(out=en[:], in_=pps[0:1, :], func=AF.Exp,
                             scale=-1.0)
        en1 = ptile("ben1", "lns1", 4, (1, S))
        nc.vector.tensor_scalar_add(out=en1[:], in0=en[:], scalar1=1.0)
        nc.vector.reciprocal(out=out_sb[0:1, 0:S], in_=en1[:])
        # bce = relu(p) - p*y + softplus(-|p|); softplus(x) = ln(1+exp(x))
        ab = ptile("bab", "lns1", 4, (1, S))
        nc.scalar.activation(out=ab[:], in_=pps[0:1, :], func=AF.Abs)
        eab = ptile("beab", "lns1", 4, (1, S))
        nc.scalar.activation(out=eab[:], in_=ab[:], func=AF.Exp, scale=-1.0)
        sp = ptile("bsp", "lns1", 4, (1, S))
        one_ap = nc.const_aps.aps[(FP, 1.0)]
        nc.scalar.activation(out=sp[:], in_=eab[:], func=AF.Ln,
                             bias=one_ap[0:1, 0:1])
        rl = ptile("brl", "lns1", 4, (1, S))
        nc.scalar.activation(out=rl[:], in_=pps[0:1, :], func=AF.Relu)
        py = ptile("bpy", "lns1", 4, (1, S))
        nc.vector.tensor_tensor(out=py[:], in0=pps[0:1, :], in1=tgt_sb[:],
                                op=ALU.mult)
        t1 = ptile("bt1", "lns1", 4, (1, S))
        nc.vector.tensor_tensor(out=t1[:], in0=rl[:], in1=py[:],
                                op=ALU.subtract)
        nc.vector.tensor_tensor(out=out_sb[0:1, S:2 * S], in0=t1[:],
                                in1=sp[:], op=ALU.add)
        nc.sync.dma_start(out=out_h[0:1, :], in_=out_sb[0:1, 0:S])
        nc.sync.dma_start(out=out_h[1:2, :], in_=out_sb[0:1, S:2 * S])

    nc.finalize()
    return nc


def _softplus(x):
    return np.logaddexp(0.0, x)


def kernel(q_data, qa_data, target, graph, params):
    global LAST_RESULT
    q_data = np.asarray(q_data, np.int32)
    qa_data = np.asarray(qa_data, np.int32)
    target = np.asarray(target, np.float32)
    graph = np.asarray(graph, np.int32)

    def npf(a):
        return np.ascontiguousarray(np.asarray(a, np.float32))

    sets = [params["b1_0"], params["b2_0"], params["b2_1"]]
    idx = np.arange(S)
    msk1 = np.where(idx[None, :] <= idx[:, None], 0.0, NEG).astype(np.float32)
    msk0 = np.where(idx[None, :] < idx[:, None], 0.0, NEG).astype(np.float32)
    posabs = np.abs(idx[None, :] - idx[:, None]).astype(np.float32)
    gneg = np.zeros((P, 24), np.float32)
    for si, ps in enumerate(sets):
        g = -_softplus(np.asarray(ps["gammas"], np.float32).reshape(H))
        gneg[:, si * 8:(si + 1) * 8] = g[None, :]

    common = {
        "q_embed": npf(params["q_embed"]),
        "qa_embed": npf(params["qa_embed"]),
        "graph_embed": npf(params["graph_embed"]),
        "posT": npf(np.asarray(params["pos_embed"], np.float32)[:S].T),
        "mask1": msk1, "mask0": msk0, "posabs": posabs,
        "fc1": npf(params["fc1_w"]), "fc2": npf(params["fc2_w"]),
        "fc3": npf(params["fc3_w"]), "gneg": gneg,
    }
    for si, ps in enumerate(sets):
        common[f"wq{si}"] = npf(np.asarray(ps["q_w"]) / math.sqrt(DK))
        common[f"wk{si}"] = npf(ps["k_w"])
        common[f"wv{si}"] = npf(ps["v_w"])
        common[f"wo{si}"] = npf(ps["o_w"])
    for si in (0, 2):
        common[f"w1{si}"] = npf(sets[si]["l1_w"])
        common[f"w2{si}"] = npf(sets[si]["l2_w"])

    r_all = ((qa_data - q_data) // NQ).astype(np.int32)
    in_maps = []
    for b in range(NCORES):
        m = dict(common)
        m["q_idx"] = np.ascontiguousarray(q_data[b].reshape(S, 1))
        m["r_idx"] = np.ascontiguousarray(r_all[b].reshape(S, 1))
        m["g_idx"] = np.ascontiguousarray(graph[:, b].reshape(S, 1))
        m["tgt"] = np.ascontiguousarray(target[b].reshape(1, S))
        in_maps.append(m)

    if "nc" not in _NC_CACHE:
        _NC_CACHE["nc"] = _build()
    nc = _NC_CACHE["nc"]

    LAST_RESULT = run_bass_kernel_spmd(nc, in_maps,
                                       core_ids=list(range(NCORES)))
    outs = [r["out"] for r in LAST_RESULT.results]
    sig = np.concatenate([o[0] for o in outs]).astype(np.float32)
    bce = np.concatenate([o[1] for o in outs]).astype(np.float64)
    labels = target.reshape(-1)
    mask = labels > -0.9
    loss = np.float32(np.sum(np.where(mask, bce, 0.0), dtype=np.float64))
    return loss, sig, np.int32(np.sum(mask))
